# revision 1
# baseline (speedup 1.0000x reference)
"""Trainium2 Bass kernel for metapath-GRU + GAT-style edge softmax message passing.

Strategy (8 NeuronCores, SPMD, no collectives):
  - Host: sort edges by destination node; core k owns nodes [2500k, 2500k+2500).
    Each core's nodes are split into 20 windows of <=128 nodes. Edges of a
    window are padded to T tiles of 128 edge slots (T = max over windows).
    Features for the 3 metapath hops are pre-gathered AND pre-transposed on
    host into xT [192, S] per core (zero for pad slots); one-hot scatter
    matrices oh [20*T, 128, 128] map edge slots -> window-local node id
    (all-zero column for pad slots).
  - Device per core: GRU over 3 steps in hid-major layout ([128 gate/hid dims,
    cw edges] tiles, fp32r matmuls, PSUM accumulate i+h gates), attention
    logits via block-diag attn matmul, leaky-relu + exp, PE-transpose back to
    edge-major, ea-weighted message scatter-matmul (one-hot) accumulated in
    PSUM per window, then divide by scattered denominator and DMA out.
  - Output: concat core shards [2500, 512] -> [20000, 8, 64].
"""

import sys

sys.path.insert(0, "/opt/trn_rl_repo")

import numpy as np

# ---- problem constants (hardcoded per contract) ----
N_NODES = 20000
N_EDGES = 100000
MP_LEN = 3
OUT_DIM = 64
NUM_HEADS = 8
HID = 512
G3 = 1536
NCORES = 8
NPC = N_NODES // NCORES          # 2500 nodes per core
WPC = (NPC + 127) // 128         # 20 windows per core
LAST_W_ROWS = NPC - 128 * (WPC - 1)  # 68

_CACHE = {}


def _split_piece(tot):
    """Split a window's T*128 edge slots into matmul pieces of 256..512."""
    pieces, rem = [], tot
    while rem > 768:
        pieces.append(512)
        rem -= 512
    if rem > 512:
        pieces += [rem - 256, 256]
    elif rem > 0:
        pieces.append(rem)
    off, out = 0, []
    for p in pieces:
        out.append((off, p))
        off += p
    return out


def _build_program(T):
    import concourse.bacc as bacc
    import concourse.tile as tile
    from concourse import mybir

    f32 = mybir.dt.float32
    f32r = mybir.dt.float32r
    AF = mybir.ActivationFunctionType
    OP = mybir.AluOpType

    S = WPC * T * 128

    nc = bacc.Bacc(
        "TRN2", target_bir_lowering=False, debug=False,
        enable_asserts=False, num_devices=NCORES,
    )
    xT = nc.dram_tensor("xT", [192, S], f32r, kind="ExternalInput").ap()
    dstloc = nc.dram_tensor("dstloc", [WPC * T, 128, 1], f32, kind="ExternalInput").ap()
    iota_d = nc.dram_tensor("iota", [128, 128], f32, kind="ExternalInput").ap()
    wihT_d = nc.dram_tensor("wihT", [64, G3], f32r, kind="ExternalInput").ap()
    whh_d = nc.dram_tensor("whh", [128, 4 * G3], f32r, kind="ExternalInput").ap()
    amat_d = nc.dram_tensor("amat", [128, 32], f32r, kind="ExternalInput").ap()
    bias_d = nc.dram_tensor("bias", [128, 16], f32, kind="ExternalInput").ap()
    ident_d = nc.dram_tensor("ident", [128, 128], f32r, kind="ExternalInput").ap()
    out_d = nc.dram_tensor("out", [NPC, HID], f32, kind="ExternalOutput").ap()

    pieces = _split_piece(T * 128)

    from contextlib import ExitStack
    with tile.TileContext(nc) as tc, ExitStack() as es:
        cpool = es.enter_context(tc.tile_pool(name="const", bufs=1))
        wk = es.enter_context(tc.tile_pool(name="work", bufs=3))
        xp = es.enter_context(tc.tile_pool(name="xp", bufs=3))
        hp = es.enter_context(tc.tile_pool(name="hp", bufs=3))
        mp = es.enter_context(tc.tile_pool(name="mp", bufs=4))
        op_ = es.enter_context(tc.tile_pool(name="op", bufs=2))
        pg = es.enter_context(tc.tile_pool(name="pg", bufs=1, space="PSUM"))
        pt = es.enter_context(tc.tile_pool(name="pt", bufs=2, space="PSUM"))
        pacc = es.enter_context(tc.tile_pool(name="pacc", bufs=1, space="PSUM"))

        wihT = cpool.tile([64, G3], f32r, name="wihT_sb")
        nc.sync.dma_start(out=wihT[:, :], in_=wihT_d[:, :])
        whh = cpool.tile([128, 4 * G3], f32r, name="whh_sb")
        nc.sync.dma_start(out=whh[:, :], in_=whh_d[:, :])
        amat = cpool.tile([128, 32], f32r, name="amat_sb")
        nc.sync.dma_start(out=amat[:, :], in_=amat_d[:, :])
        bias = cpool.tile([128, 16], f32, name="bias_sb")
        nc.sync.dma_start(out=bias[:, :], in_=bias_d[:, :])
        ident = cpool.tile([128, 128], f32r, name="ident_sb")
        nc.sync.dma_start(out=ident[:, :], in_=ident_d[:, :])
        iota = cpool.tile([128, 128], f32, name="iota_sb")
        nc.sync.dma_start(out=iota[:, :], in_=iota_d[:, :])

        def b_r(j):
            return bias[:, j:j + 1]

        def b_z(j):
            return bias[:, 4 + j:5 + j]

        def b_in(j):
            return bias[:, 8 + j:9 + j]

        def b_hn(j):
            return bias[:, 12 + j:13 + j]

        def wih_slice(gate, j):
            o = gate * HID + j * 128
            return wihT[:, o:o + 128]

        def whh_slice(k, gate, j):
            o = k * G3 + gate * HID + j * 128
            return whh[:, o:o + 128]

        for w in range(WPC):
            rows = 128 if w < WPC - 1 else LAST_W_ROWS
            macc = pacc.tile([128, HID], f32, name=f"macc{w}", tag="macc")
            dacc = pacc.tile([128, 8], f32, name=f"dacc{w}", tag="dacc")
            n_et_total = T
            et_done = 0
            for (off, cw) in pieces:
                base = w * T * 128 + off
                # ---- load x for 3 steps ----
                xs = []
                for t in range(3):
                    xt = xp.tile([64, cw], f32r, name=f"x{w}_{off}_{t}", tag=f"x{t}")
                    nc.sync.dma_start(out=xt[:, :], in_=xT[t * 64:(t + 1) * 64, base:base + cw])
                    xs.append(xt)
                # ---- GRU ----
                h_cur = [None] * 4
                for step in range(3):
                    xt = xs[step][:, :]
                    h_new = []
                    for j in range(4):
                        psr = pg.tile([128, cw], f32, name=f"psr{w}{off}{step}{j}", tag="r")
                        psz = pg.tile([128, cw], f32, name=f"psz{w}{off}{step}{j}", tag="z")
                        psn = pg.tile([128, cw], f32, name=f"psn{w}{off}{step}{j}", tag="nn")
                        if step == 0:
                            nc.tensor.matmul(psr[:, :], wih_slice(0, j), xt, start=True, stop=True)
                            nc.tensor.matmul(psz[:, :], wih_slice(1, j), xt, start=True, stop=True)
                            nc.tensor.matmul(psn[:, :], wih_slice(2, j), xt, start=True, stop=True)
                        else:
                            nc.tensor.matmul(psr[:, :], wih_slice(0, j), xt, start=True, stop=False)
                            nc.tensor.matmul(psz[:, :], wih_slice(1, j), xt, start=True, stop=False)
                            for k in range(4):
                                hk = h_cur[k][:, :]
                                nc.tensor.matmul(psr[:, :], whh_slice(k, 0, j), hk,
                                                 start=False, stop=(k == 3))
                                nc.tensor.matmul(psz[:, :], whh_slice(k, 1, j), hk,
                                                 start=False, stop=(k == 3))
                            nc.tensor.matmul(psn[:, :], wih_slice(2, j), xt, start=True, stop=True)
                            pshn = pg.tile([128, cw], f32, name=f"pshn{w}{off}{step}{j}", tag="hn")
                            for k in range(4):
                                nc.tensor.matmul(pshn[:, :], whh_slice(k, 2, j),
                                                 h_cur[k][:, :],
                                                 start=(k == 0), stop=(k == 3))
                        r_sb = wk.tile([128, cw], f32, name=f"r{w}{off}{step}{j}", tag="r_sb")
                        z_sb = wk.tile([128, cw], f32, name=f"z{w}{off}{step}{j}", tag="z_sb")
                        nc.scalar.activation(r_sb[:, :], psr[:, :], AF.Sigmoid, bias=b_r(j))
                        nc.scalar.activation(z_sb[:, :], psz[:, :], AF.Sigmoid, bias=b_z(j))
                        t1 = wk.tile([128, cw], f32, name=f"t1{w}{off}{step}{j}", tag="t1")
                        if step == 0:
                            nc.vector.tensor_scalar(t1[:, :], r_sb[:, :], b_hn(j), None, op0=OP.mult)
                        else:
                            hn_sb = wk.tile([128, cw], f32, name=f"hn{w}{off}{step}{j}", tag="hn_sb")
                            nc.vector.tensor_scalar(hn_sb[:, :], pshn[:, :], b_hn(j), None, op0=OP.add)
                            nc.vector.tensor_tensor(t1[:, :], r_sb[:, :], hn_sb[:, :], op=OP.mult)
                        t2 = wk.tile([128, cw], f32, name=f"t2{w}{off}{step}{j}", tag="t2")
                        nc.vector.tensor_tensor(t2[:, :], psn[:, :], t1[:, :], op=OP.add)
                        n_sb = wk.tile([128, cw], f32, name=f"n{w}{off}{step}{j}", tag="n_sb")
                        nc.scalar.activation(n_sb[:, :], t2[:, :], AF.Tanh, bias=b_in(j))
                        ho = hp.tile([128, cw], f32r, name=f"h{w}{off}{step}{j}",
                                     tag=f"h{step % 2}{j}")
                        t3 = wk.tile([128, cw], f32, name=f"t3{w}{off}{step}{j}", tag="t3")
                        if step == 0:
                            nc.vector.tensor_tensor(t3[:, :], z_sb[:, :], n_sb[:, :], op=OP.mult)
                            nc.vector.tensor_tensor(ho[:, :], n_sb[:, :], t3[:, :], op=OP.subtract)
                        else:
                            d_sb = wk.tile([128, cw], f32, name=f"d{w}{off}{step}{j}", tag="d_sb")
                            nc.vector.tensor_tensor(d_sb[:, :], h_cur[j][:, :], n_sb[:, :], op=OP.subtract)
                            nc.vector.tensor_tensor(t3[:, :], z_sb[:, :], d_sb[:, :], op=OP.mult)
                            nc.vector.tensor_tensor(ho[:, :], n_sb[:, :], t3[:, :], op=OP.add)
                        h_new.append(ho)
                    h_cur = h_new
                # ---- attention logits: aT [8, cw] ----
                psa = pg.tile([8, cw], f32, name=f"psa{w}{off}", tag="nn")
                for k in range(4):
                    nc.tensor.matmul(psa[:, :], amat[:, k * 8:(k + 1) * 8],
                                     h_cur[k][:, :], start=(k == 0), stop=(k == 3))
                # leaky relu on DVE (exact semantics), then exp on ACT
                lr_a = wk.tile([8, cw], f32, name=f"lra{w}{off}", tag="lra")
                lr_b = wk.tile([8, cw], f32, name=f"lrb{w}{off}", tag="lrb")
                nc.vector.tensor_scalar(lr_a[:, :], psa[:, :], 0.0, 0.01, op0=OP.min, op1=OP.mult)
                nc.vector.tensor_scalar(lr_b[:, :], psa[:, :], 0.0, None, op0=OP.max)
                lr = wk.tile([8, cw], f32, name=f"lr{w}{off}", tag="lr")
                nc.vector.tensor_tensor(lr[:, :], lr_a[:, :], lr_b[:, :], op=OP.add)
                th = wk.tile([8, cw], f32, name=f"th{w}{off}", tag="th")
                nc.scalar.activation(th[:, :], lr[:, :], AF.Tanh, scale=0.5)
                enm = wk.tile([8, cw], f32, name=f"enm{w}{off}", tag="enm")
                nc.vector.tensor_scalar(enm[:, :], th[:, :], 1.0, None, op0=OP.add)
                edn = wk.tile([8, cw], f32, name=f"edn{w}{off}", tag="edn")
                nc.vector.tensor_scalar(edn[:, :], th[:, :], -1.0, 1.0, op0=OP.mult, op1=OP.add)
                erc = wk.tile([8, cw], f32, name=f"erc{w}{off}", tag="erc")
                nc.vector.reciprocal(erc[:, :], edn[:, :])
                eaT = wk.tile([8, cw], f32r, name=f"eaT{w}{off}", tag="eaT")
                nc.vector.tensor_tensor(eaT[:, :], enm[:, :], erc[:, :], op=OP.mult)
                # ---- per e-tile: transpose, ea-mul, scatter ----
                for et in range(cw // 128):
                    ti = w * T + (off // 128) + et
                    es = et * 128
                    # ea -> edge-major [128, 8]
                    pse = pt.tile([128, 8], f32r, name=f"pse{ti}", tag="tp")
                    nc.tensor.transpose(pse[:, :], eaT[:, es:es + 128], ident[:8, :8])
                    ea_em = mp.tile([128, 8], f32r, name=f"eaem{ti}", tag="ea_em")
                    nc.scalar.activation(ea_em[:, :], pse[:, :], AF.Copy)
                    # msg edge-major [128, 512], scaled by ea per head
                    msg = mp.tile([128, HID], f32r, name=f"msg{ti}", tag="msg")
                    for j in range(4):
                        pst = pt.tile([128, 128], f32r, name=f"pst{ti}{j}", tag="tp")
                        nc.tensor.transpose(pst[:, :], h_cur[j][:, es:es + 128], ident[:, :])
                        for hh in range(2):
                            hd = 2 * j + hh
                            nc.vector.tensor_scalar(
                                msg[:, hd * 64:(hd + 1) * 64], pst[:, hh * 64:(hh + 1) * 64],
                                ea_em[:, hd:hd + 1].bitcast(f32), None, op0=OP.mult)
                    # scatter via one-hot matmul, accumulate over window
                    dl = mp.tile([128, 1], f32, name=f"dl{ti}", tag="dl")
                    nc.sync.dma_start(out=dl[:, :], in_=dstloc[ti])
                    ohs = mp.tile([128, 128], f32r, name=f"ohs{ti}", tag="ohs")
                    nc.vector.tensor_scalar(ohs[:, :], iota[:, :], dl[:, :1], None, op0=OP.is_equal)
                    first = (et_done == 0)
                    last = (et_done == n_et_total - 1)
                    nc.tensor.matmul(macc[:, :], ohs[:, :], msg[:, :],
                                     start=first, stop=last, skip_group_check=True)
                    nc.tensor.matmul(dacc[:, :], ohs[:, :], ea_em[:, :],
                                     start=first, stop=last, skip_group_check=True)
                    et_done += 1
            # ---- finalize window: out = macc / max(dacc, eps) ----
            dmax = op_.tile([128, 8], f32, name=f"dmax{w}", tag="dmax")
            nc.vector.tensor_scalar(dmax[:, :], dacc[:, :], 1e-30, None, op0=OP.max)
            rec = op_.tile([128, 8], f32, name=f"rec{w}", tag="rec")
            nc.vector.reciprocal(rec[:, :], dmax[:, :])
            osb = op_.tile([128, HID], f32, name=f"osb{w}", tag="osb")
            for hd in range(8):
                nc.vector.tensor_scalar(osb[:, hd * 64:(hd + 1) * 64],
                                        macc[:, hd * 64:(hd + 1) * 64],
                                        rec[:, hd:hd + 1], None, op0=OP.mult)
            nc.sync.dma_start(out=out_d[w * 128:w * 128 + rows, :], in_=osb[:rows, :])

    nc.compile()
    return nc


def _preprocess(features, W_ih, W_hh, b_ih, b_hh, attn, idx, dst):
    feats = np.asarray(features, np.float32)
    idx = np.asarray(idx).astype(np.int64)
    dst = np.asarray(dst).astype(np.int64)
    order = np.argsort(dst, kind="stable")
    ds = dst[order]
    idxs = idx[order]
    core_of = ds // NPC
    local = ds % NPC
    win = local // 128
    nloc = local % 128
    wgid = core_of * WPC + win
    cnt = np.bincount(wgid, minlength=NCORES * WPC)
    T = int(np.ceil(cnt.max() / 128.0))
    S = WPC * T * 128
    start = np.zeros(NCORES * WPC, np.int64)
    start[1:] = np.cumsum(cnt)[:-1]
    rank = np.arange(N_EDGES) - start[wgid]
    core_slot = (wgid - core_of * WPC) * (T * 128) + rank
    g = feats[idxs]  # [E, 3, 64]
    xT_all = np.zeros((NCORES, 192, S), np.float32)
    xT_all[core_of, :, core_slot] = g.reshape(N_EDGES, 192)
    dl_all = np.full((NCORES, WPC * T, 128, 1), 200.0, np.float32)
    dl_all[core_of, core_slot // 128, core_slot % 128, 0] = nloc

    W_ih = np.asarray(W_ih, np.float32)
    W_hh = np.asarray(W_hh, np.float32)
    b_ih = np.asarray(b_ih, np.float32)
    b_hh = np.asarray(b_hh, np.float32)
    attn = np.asarray(attn, np.float32)
    wihT = np.ascontiguousarray(W_ih.T)  # [64, 1536]
    whhT = W_hh.T  # [512, 1536]
    whh6 = np.concatenate([whhT[k * 128:(k + 1) * 128, :] for k in range(4)], axis=1)
    b_rz = b_ih + b_hh
    bias16 = np.zeros((128, 16), np.float32)
    for j in range(4):
        bias16[:, j] = b_rz[j * 128:(j + 1) * 128]
        bias16[:, 4 + j] = b_rz[HID + j * 128:HID + (j + 1) * 128]
        bias16[:, 8 + j] = b_ih[2 * HID + j * 128:2 * HID + (j + 1) * 128]
        bias16[:, 12 + j] = b_hh[2 * HID + j * 128:2 * HID + (j + 1) * 128]
    amat = np.zeros((HID, 8), np.float32)
    for h in range(8):
        amat[h * 64:(h + 1) * 64, h] = attn[h]
    amat32 = np.zeros((128, 32), np.float32)
    for k in range(4):
        amat32[:, k * 8:(k + 1) * 8] = amat[k * 128:(k + 1) * 128, :]
    ident = np.eye(128, dtype=np.float32)
    iota = np.tile(np.arange(128, dtype=np.float32)[None, :], (128, 1))
    shared = dict(wihT=np.ascontiguousarray(wihT),
                  whh=np.ascontiguousarray(whh6),
                  amat=amat32, bias=bias16, ident=ident, iota=iota)
    in_maps = []
    for c in range(NCORES):
        m = dict(shared)
        m["xT"] = np.ascontiguousarray(xT_all[c])
        m["dstloc"] = np.ascontiguousarray(dl_all[c])
        in_maps.append(m)
    return T, in_maps


def kernel(**inputs):
    from concourse.bass_utils import run_bass_kernel_spmd

    T, in_maps = _preprocess(
        inputs["features"], inputs["W_ih"], inputs["W_hh"], inputs["b_ih"],
        inputs["b_hh"], inputs["attn"], inputs["edge_metapath_indices"],
        inputs["edge_dst"])
    if T not in _CACHE:
        _CACHE[T] = _build_program(T)
    nc = _CACHE[T]
    res = run_bass_kernel_spmd(nc, in_maps, core_ids=list(range(NCORES)))
    out = np.concatenate([res.results[c]["out"] for c in range(NCORES)], axis=0)
    return out.reshape(N_NODES, NUM_HEADS, OUT_DIM).astype(np.float32)


if __name__ == "__main__":
    rng = np.random.default_rng(0)
    pass



# revision 2
# speedup vs baseline: 1.1649x; 1.1649x over previous
"""Trainium2 Bass kernel for metapath-GRU + GAT-style edge softmax message passing.

v3 — instruction-count optimized. This runtime charges ~60-130us per
instruction on the critical path, so fewer/wider instructions dominate wall
time; transfer bytes come second (~50MB/s tunnel).

  - Features uploaded SHARDED fp16, AllGathered on device, relayed into a
    256B-stride padded table [20480,128] fp16; per-window dma_gather (SWDGE,
    <=58 descriptors per instruction to fit the runtime's descriptor ring)
    pulls edge-slot features feature-major into SBUF.
  - Window-PAIR gather tiles [128, 2*T*128]: two 768-slot gathers per hop land
    in one tile so the GRU runs contiguous 512-wide pieces (30/core instead of
    40 window-aligned ones).
  - One PSUM bank holds the softmax denominators of ALL windows
    ([128, 8*WPC], window w in cols 8w:8w+8); numerator macc rotates 2 banks.
  - Weights: whh+smalls fused into one f32 AllGather; wihT replicated (tiny).
  - Output fp16 [2500,512] per core.
"""

import sys

sys.path.insert(0, "/opt/trn_rl_repo")

import numpy as np

# ---- problem constants (hardcoded per contract) ----
N_NODES = 20000
N_EDGES = 100000
MP_LEN = 3
OUT_DIM = 64
NUM_HEADS = 8
HID = 512
G3 = 1536
NCORES = 8
NPC = N_NODES // NCORES          # 2500 nodes per core
WPC = (NPC + 127) // 128         # 20 windows per core
LAST_W_ROWS = NPC - 128 * (WPC - 1)  # 68
NPAD = 20480                     # feature table padded to 160*128 rows
SHARD = NPAD // NCORES           # 2560

_CACHE = {}


def _segc(T):
    """idx columns per (hop,window) segment, padded to 32-col (64B) alignment."""
    return ((T * 8 + 31) // 32) * 32


def _split_piece(tot):
    pieces, rem = [], tot
    while rem > 768:
        pieces.append(512)
        rem -= 512
    if rem > 512:
        pieces += [rem - 256, 256]
    elif rem > 0:
        pieces.append(rem)
    off, out = 0, []
    for p in pieces:
        out.append((off, p))
        off += p
    return out


def _build_program(T):
    import concourse.bacc as bacc
    import concourse.tile as tile
    from concourse import mybir
    from concourse import library_config

    f32 = mybir.dt.float32
    f32r = mybir.dt.float32r
    f16 = mybir.dt.float16
    i16 = mybir.dt.int16
    AF = mybir.ActivationFunctionType
    OP = mybir.AluOpType

    S = WPC * T * 128               # edge slots per core
    WS = T * 128                    # slots per window
    SEGC = _segc(T)                 # idx cols per (hop,window) segment
    assert T <= 7, "per-window gather would exceed the SWDGE descriptor ring"
    # group = 2 windows when the pair is 512-divisible (even T), else 1
    GRP = 2 if (2 * WS) % 512 == 0 else 1
    GS = GRP * WS
    NGRP = WPC // GRP
    gpieces = ([(i * 512, 512) for i in range(GS // 512)]
               if GS % 512 == 0 else _split_piece(GS))

    nc = bacc.Bacc(
        "TRN2", target_bir_lowering=False, debug=False,
        enable_asserts=False, num_devices=NCORES,
    )
    fshard_d = nc.dram_tensor("fshard", [SHARD, 64], f16, kind="ExternalInput").ap()
    idx_d = nc.dram_tensor("idx", [16, 3 * WPC * SEGC], i16, kind="ExternalInput").ap()
    dstloc = nc.dram_tensor("dstloc", [WPC * T, 128, 1], f32, kind="ExternalInput").ap()
    wihT_d = nc.dram_tensor("wihT", [64, G3], f16, kind="ExternalInput").ap()
    # wsm: [16, 6448] f32 shard of [128, 6448] = whh(6144) | amat32(32) |
    # bias16(16) | iota(128) | ident(128)
    wsm_sh = nc.dram_tensor("wsm", [16, 4 * G3 + 304], f32, kind="ExternalInput").ap()
    out_d = nc.dram_tensor("out", [NPC, HID], f16, kind="ExternalOutput").ap()

    from contextlib import ExitStack
    with tile.TileContext(nc) as tc, ExitStack() as es:
        nc.gpsimd.load_library(library_config.mlp)

        dram = es.enter_context(tc.tile_pool(name="dram", bufs=1, space="DRAM"))
        cpool = es.enter_context(tc.tile_pool(name="const", bufs=1))
        gp = es.enter_context(tc.tile_pool(name="gp", bufs=2))
        wk = es.enter_context(tc.tile_pool(name="work", bufs=2))
        hp = es.enter_context(tc.tile_pool(name="hp", bufs=2))
        mp = es.enter_context(tc.tile_pool(name="mp", bufs=4))
        op_ = es.enter_context(tc.tile_pool(name="op", bufs=2))
        pg = es.enter_context(tc.tile_pool(name="pg", bufs=1, space="PSUM"))
        pt = es.enter_context(tc.tile_pool(name="pt", bufs=1, space="PSUM"))
        pam = es.enter_context(tc.tile_pool(name="pam", bufs=2, space="PSUM"))
        pad_ = es.enter_context(tc.tile_pool(name="pad", bufs=1, space="PSUM"))

        # ---- collectives ----
        rg = [list(range(NCORES))]
        BP = mybir.AluOpType.bypass

        fb_in = dram.tile([SHARD, 64], f16, name="fb_in")
        fb_out = dram.tile([NPAD, 64], f16, name="fb_out")
        padded = dram.tile([NPAD, 128], f16, name="padded")
        nc.sync.dma_start(out=fb_in[:, :], in_=fshard_d[:, :])
        nc.gpsimd.collective_compute(
            "AllGather", BP, replica_groups=rg,
            ins=[fb_in[:, :].opt()], outs=[fb_out[:, :].opt()])

        wsm_in = dram.tile([16, 4 * G3 + 304], f32, name="wsm_in")
        wsm_out = dram.tile([128, 4 * G3 + 304], f32, name="wsm_out")
        nc.sync.dma_start(out=wsm_in[:, :], in_=wsm_sh[:, :])
        nc.gpsimd.collective_compute(
            "AllGather", BP, replica_groups=rg,
            ins=[wsm_in[:, :].opt()], outs=[wsm_out[:, :].opt()])

        # ---- feature relayout: [NPAD,64] -> padded[:,0:64] (256B stride) ----
        NB = NPAD // 128
        for b in range(NB):
            nc.sync.dma_start(out=padded[b * 128:(b + 1) * 128, 0:64],
                              in_=fb_out[b * 128:(b + 1) * 128, :])

        # ---- weights into SBUF ----
        wihT = cpool.tile([64, G3], f16, name="wihT_sb")
        nc.sync.dma_start(out=wihT[:, :], in_=wihT_d[:, :])
        whh = cpool.tile([128, 4 * G3], f32r, name="whh_sb")
        nc.sync.dma_start(out=whh[:, :], in_=wsm_out[:, 0:4 * G3].bitcast(f32r))
        smalls = cpool.tile([128, 304], f32, name="smalls_sb")
        nc.sync.dma_start(out=smalls[:, :], in_=wsm_out[:, 4 * G3:4 * G3 + 304])
        amat = smalls[:, 0:32].bitcast(f32r)
        bias = smalls[:, 32:48]
        iota = smalls[:, 48:176]
        ident = smalls[:, 176:304].bitcast(f32r)

        # ---- indices into SBUF (8x replicated over 16-partition groups) ----
        idxs = cpool.tile([128, 3 * WPC * SEGC], i16, name="idx_sb")
        for k in range(8):
            nc.sync.dma_start(out=idxs[16 * k:16 * (k + 1), :], in_=idx_d[:, :])

        # denominators for ALL windows live in one PSUM bank
        dacc = pad_.tile([128, 8 * WPC], f32, name="dacc_all")

        def gather_group(gi):
            """Gather group gi's windows (GRP of them) for all 3 hops."""
            gt3 = []
            for t in range(3):
                gt = gp.tile([128, 1, GS], f16, name=f"g{t}_{gi}", tag=f"g{t}")
                for h in range(GRP):
                    w = gi * GRP + h
                    i0 = (t * WPC + w) * SEGC
                    nc.gpsimd.dma_gather(
                        gt[:, :, h * WS:(h + 1) * WS], padded[:, :],
                        idxs[:, i0:i0 + WS // 16],
                        WS, WS, 128, transpose=True)
                gt3.append(gt)
            return gt3

        def b_r(j):
            return bias[:, j:j + 1]

        def b_z(j):
            return bias[:, 4 + j:5 + j]

        def b_in(j):
            return bias[:, 8 + j:9 + j]

        def b_hn(j):
            return bias[:, 12 + j:13 + j]

        def wih_slice(gate, j):
            o = gate * HID + j * 128
            return wihT[:, o:o + 128]

        def whh_slice(k, gate, j):
            o = k * G3 + gate * HID + j * 128
            return whh[:, o:o + 128]

        macc_of = {}

        def finalize_window(w):
            rows = 128 if w < WPC - 1 else LAST_W_ROWS
            macc = macc_of.pop(w)
            dmax = op_.tile([128, 8], f32, name=f"dmax{w}", tag="dmax")
            nc.vector.tensor_scalar(dmax[:, :], dacc[:, 8 * w:8 * w + 8],
                                    1e-30, None, op0=OP.max)
            rec = op_.tile([128, 8], f32, name=f"rec{w}", tag="rec")
            nc.vector.reciprocal(rec[:, :], dmax[:, :])
            osb = op_.tile([128, HID], f16, name=f"osb{w}", tag="osb")
            for hd in range(8):
                nc.vector.tensor_scalar(osb[:, hd * 64:(hd + 1) * 64],
                                        macc[:, hd * 64:(hd + 1) * 64],
                                        rec[:, hd:hd + 1], None, op0=OP.mult)
            nc.sync.dma_start(out=out_d[w * 128:w * 128 + rows, :], in_=osb[:rows, :])

        for gi in range(NGRP):
            gcur = gather_group(gi)
            for (off, cw) in gpieces:
                xs = [gcur[t][0:64, 0, off:off + cw] for t in range(3)]
                # ---- GRU ----
                h_cur = [None] * 4
                for step in range(3):
                    xt = xs[step]
                    h_new = []
                    for j in range(4):
                        psr = pg.tile([128, cw], f32, name=f"psr{gi}{off}{step}{j}", tag="r")
                        psz = pg.tile([128, cw], f32, name=f"psz{gi}{off}{step}{j}", tag="z")
                        psn = pg.tile([128, cw], f32, name=f"psn{gi}{off}{step}{j}", tag="nn")
                        if step == 0:
                            nc.tensor.matmul(psr[:, :], wih_slice(0, j), xt, start=True, stop=True)
                            nc.tensor.matmul(psz[:, :], wih_slice(1, j), xt, start=True, stop=True)
                            nc.tensor.matmul(psn[:, :], wih_slice(2, j), xt, start=True, stop=True)
                        else:
                            nc.tensor.matmul(psr[:, :], wih_slice(0, j), xt, start=True, stop=False)
                            nc.tensor.matmul(psz[:, :], wih_slice(1, j), xt, start=True, stop=False)
                            for k in range(4):
                                hk = h_cur[k][:, :]
                                nc.tensor.matmul(psr[:, :], whh_slice(k, 0, j), hk,
                                                 start=False, stop=(k == 3))
                                nc.tensor.matmul(psz[:, :], whh_slice(k, 1, j), hk,
                                                 start=False, stop=(k == 3))
                            nc.tensor.matmul(psn[:, :], wih_slice(2, j), xt, start=True, stop=True)
                            pshn = pg.tile([128, cw], f32, name=f"pshn{gi}{off}{step}{j}", tag="hn")
                            for k in range(4):
                                nc.tensor.matmul(pshn[:, :], whh_slice(k, 2, j),
                                                 h_cur[k][:, :],
                                                 start=(k == 0), stop=(k == 3))
                        r_sb = wk.tile([128, cw], f32, name=f"r{gi}{off}{step}{j}", tag="r_sb")
                        z_sb = wk.tile([128, cw], f32, name=f"z{gi}{off}{step}{j}", tag="z_sb")
                        nc.scalar.activation(r_sb[:, :], psr[:, :], AF.Sigmoid, bias=b_r(j))
                        nc.scalar.activation(z_sb[:, :], psz[:, :], AF.Sigmoid, bias=b_z(j))
                        t1 = wk.tile([128, cw], f32, name=f"t1{gi}{off}{step}{j}", tag="t1")
                        if step == 0:
                            nc.vector.tensor_scalar(t1[:, :], r_sb[:, :], b_hn(j), None, op0=OP.mult)
                        else:
                            hn_sb = wk.tile([128, cw], f32, name=f"hn{gi}{off}{step}{j}", tag="hn_sb")
                            nc.vector.tensor_scalar(hn_sb[:, :], pshn[:, :], b_hn(j), None, op0=OP.add)
                            nc.vector.tensor_tensor(t1[:, :], r_sb[:, :], hn_sb[:, :], op=OP.mult)
                        t2 = wk.tile([128, cw], f32, name=f"t2{gi}{off}{step}{j}", tag="t2")
                        nc.vector.tensor_tensor(t2[:, :], psn[:, :], t1[:, :], op=OP.add)
                        n_sb = wk.tile([128, cw], f32, name=f"n{gi}{off}{step}{j}", tag="n_sb")
                        nc.scalar.activation(n_sb[:, :], t2[:, :], AF.Tanh, bias=b_in(j))
                        ho = hp.tile([128, cw], f32r, name=f"h{gi}{off}{step}{j}",
                                     tag=f"h{step % 2}{j}")
                        t3 = wk.tile([128, cw], f32, name=f"t3{gi}{off}{step}{j}", tag="t3")
                        if step == 0:
                            nc.vector.tensor_tensor(t3[:, :], z_sb[:, :], n_sb[:, :], op=OP.mult)
                            nc.vector.tensor_tensor(ho[:, :], n_sb[:, :], t3[:, :], op=OP.subtract)
                        else:
                            d_sb = wk.tile([128, cw], f32, name=f"d{gi}{off}{step}{j}", tag="d_sb")
                            nc.vector.tensor_tensor(d_sb[:, :], h_cur[j][:, :], n_sb[:, :], op=OP.subtract)
                            nc.vector.tensor_tensor(t3[:, :], z_sb[:, :], d_sb[:, :], op=OP.mult)
                            nc.vector.tensor_tensor(ho[:, :], n_sb[:, :], t3[:, :], op=OP.add)
                        h_new.append(ho)
                    h_cur = h_new
                # ---- attention logits: aT [8, cw] ----
                psa = pg.tile([8, cw], f32, name=f"psa{gi}{off}", tag="nn")
                for k in range(4):
                    nc.tensor.matmul(psa[:, :], amat[:, k * 8:(k + 1) * 8],
                                     h_cur[k][:, :], start=(k == 0), stop=(k == 3))
                lr_a = wk.tile([8, cw], f32, name=f"lra{gi}{off}", tag="lra")
                lr_b = wk.tile([8, cw], f32, name=f"lrb{gi}{off}", tag="lrb")
                nc.vector.tensor_scalar(lr_a[:, :], psa[:, :], 0.0, 0.01, op0=OP.min, op1=OP.mult)
                nc.vector.tensor_scalar(lr_b[:, :], psa[:, :], 0.0, None, op0=OP.max)
                lr = wk.tile([8, cw], f32, name=f"lr{gi}{off}", tag="lr")
                nc.vector.tensor_tensor(lr[:, :], lr_a[:, :], lr_b[:, :], op=OP.add)
                th = wk.tile([8, cw], f32, name=f"th{gi}{off}", tag="th")
                nc.scalar.activation(th[:, :], lr[:, :], AF.Tanh, scale=0.5)
                enm = wk.tile([8, cw], f32, name=f"enm{gi}{off}", tag="enm")
                nc.vector.tensor_scalar(enm[:, :], th[:, :], 1.0, None, op0=OP.add)
                edn = wk.tile([8, cw], f32, name=f"edn{gi}{off}", tag="edn")
                nc.vector.tensor_scalar(edn[:, :], th[:, :], -1.0, 1.0, op0=OP.mult, op1=OP.add)
                erc = wk.tile([8, cw], f32, name=f"erc{gi}{off}", tag="erc")
                nc.vector.reciprocal(erc[:, :], edn[:, :])
                eaT = wk.tile([8, cw], f32r, name=f"eaT{gi}{off}", tag="eaT")
                nc.vector.tensor_tensor(eaT[:, :], enm[:, :], erc[:, :], op=OP.mult)
                # ---- per e-tile: transpose, ea-mul, scatter ----
                for et in range(cw // 128):
                    ti = (gi * GS + off) // 128 + et
                    w = ti // T
                    ees = et * 128
                    if ti % T == 0:
                        macc_of[w] = pam.tile([128, HID], f32, name=f"macc{w}", tag="macc")
                    macc = macc_of[w]
                    pse = pt.tile([128, 8], f32r, name=f"pse{ti}", tag="tp")
                    nc.tensor.transpose(pse[:, :], eaT[:, ees:ees + 128], ident[:8, :8])
                    ea_em = mp.tile([128, 8], f32r, name=f"eaem{ti}", tag="ea_em")
                    nc.scalar.activation(ea_em[:, :], pse[:, :], AF.Copy)
                    msg = mp.tile([128, HID], f32r, name=f"msg{ti}", tag="msg")
                    for j in range(4):
                        pst = pt.tile([128, 128], f32r, name=f"pst{ti}{j}", tag="tp")
                        nc.tensor.transpose(pst[:, :], h_cur[j][:, ees:ees + 128], ident[:, :])
                        for hh in range(2):
                            hd = 2 * j + hh
                            nc.vector.tensor_scalar(
                                msg[:, hd * 64:(hd + 1) * 64], pst[:, hh * 64:(hh + 1) * 64],
                                ea_em[:, hd:hd + 1].bitcast(f32), None, op0=OP.mult)
                    dl = mp.tile([128, 1], f32, name=f"dl{ti}", tag="dl")
                    nc.sync.dma_start(out=dl[:, :], in_=dstloc[ti])
                    ohs = mp.tile([128, 128], f32r, name=f"ohs{ti}", tag="ohs")
                    nc.vector.tensor_scalar(ohs[:, :], iota[:, :], dl[:, :1], None, op0=OP.is_equal)
                    first = (ti % T == 0)
                    last = (ti % T == T - 1)
                    nc.tensor.matmul(macc[:, :], ohs[:, :], msg[:, :],
                                     start=first, stop=last, skip_group_check=True)
                    nc.tensor.matmul(dacc[:, 8 * w:8 * w + 8], ohs[:, :], ea_em[:, :],
                                     start=first, stop=last, skip_group_check=True)
                    if last:
                        finalize_window(w)

    nc.compile()
    return nc


def _preprocess(features, W_ih, W_hh, b_ih, b_hh, attn, idx, dst):
    feats = np.asarray(features, np.float32)
    idx = np.asarray(idx).astype(np.int64)
    dst = np.asarray(dst).astype(np.int64)
    order = np.argsort(dst, kind="stable")
    ds = dst[order]
    idxs = idx[order]
    core_of = ds // NPC
    local = ds % NPC
    win = local // 128
    nloc = local % 128
    wgid = core_of * WPC + win
    cnt = np.bincount(wgid, minlength=NCORES * WPC)
    T = int(np.ceil(cnt.max() / 128.0))
    S = WPC * T * 128
    start = np.zeros(NCORES * WPC, np.int64)
    start[1:] = np.cumsum(cnt)[:-1]
    rank = np.arange(N_EDGES) - start[wgid]
    core_slot = (wgid - core_of * WPC) * (T * 128) + rank

    # per-core (hop, window)-segmented int16 slot indices, wrapped [16, cols]
    SEGC = _segc(T)
    win_of_slot = core_slot // (T * 128)
    pos_in_win = core_slot % (T * 128)
    idx_all = np.zeros((NCORES, 3, WPC, SEGC * 16), np.int16)
    for t in range(3):
        idx_all[core_of, t, win_of_slot, pos_in_win] = idxs[:, t].astype(np.int16)
    cols = 3 * WPC * SEGC
    idx_w = np.ascontiguousarray(
        idx_all.reshape(NCORES, cols, 16).transpose(0, 2, 1))

    dl_all = np.full((NCORES, WPC * T, 128, 1), 200.0, np.float32)
    dl_all[core_of, core_slot // 128, core_slot % 128, 0] = nloc

    # padded fp16 feature table, sharded
    fpad = np.zeros((NPAD, 64), np.float16)
    fpad[:N_NODES] = feats.astype(np.float16)

    W_ih = np.asarray(W_ih, np.float32)
    W_hh = np.asarray(W_hh, np.float32)
    b_ih = np.asarray(b_ih, np.float32)
    b_hh = np.asarray(b_hh, np.float32)
    attn = np.asarray(attn, np.float32)
    wihT = np.ascontiguousarray(W_ih.T).astype(np.float16)  # [64, 1536]
    whhT = W_hh.T  # [512, 1536]
    whh6 = np.concatenate([whhT[k * 128:(k + 1) * 128, :] for k in range(4)], axis=1)
    b_rz = b_ih + b_hh
    bias16 = np.zeros((128, 16), np.float32)
    for j in range(4):
        bias16[:, j] = b_rz[j * 128:(j + 1) * 128]
        bias16[:, 4 + j] = b_rz[HID + j * 128:HID + (j + 1) * 128]
        bias16[:, 8 + j] = b_ih[2 * HID + j * 128:2 * HID + (j + 1) * 128]
        bias16[:, 12 + j] = b_hh[2 * HID + j * 128:2 * HID + (j + 1) * 128]
    amat = np.zeros((HID, 8), np.float32)
    for h in range(8):
        amat[h * 64:(h + 1) * 64, h] = attn[h]
    amat32 = np.zeros((128, 32), np.float32)
    for k in range(4):
        amat32[:, k * 8:(k + 1) * 8] = amat[k * 128:(k + 1) * 128, :]
    ident = np.eye(128, dtype=np.float32)
    iota = np.tile(np.arange(128, dtype=np.float32)[None, :], (128, 1))
    smalls = np.concatenate([amat32, bias16, iota, ident], axis=1)  # [128, 304]
    wsm = np.ascontiguousarray(
        np.concatenate([whh6.astype(np.float32), smalls], axis=1))  # [128, 6448]

    in_maps = []
    for c in range(NCORES):
        in_maps.append({
            "fshard": np.ascontiguousarray(fpad[c * SHARD:(c + 1) * SHARD]),
            "idx": np.ascontiguousarray(idx_w[c]),
            "dstloc": np.ascontiguousarray(dl_all[c]),
            "wihT": wihT,
            "wsm": np.ascontiguousarray(wsm[c * 16:(c + 1) * 16]),
        })
    return T, in_maps


def kernel(**inputs):
    from concourse.bass_utils import run_bass_kernel_spmd

    T, in_maps = _preprocess(
        inputs["features"], inputs["W_ih"], inputs["W_hh"], inputs["b_ih"],
        inputs["b_hh"], inputs["attn"], inputs["edge_metapath_indices"],
        inputs["edge_dst"])
    if T not in _CACHE:
        _CACHE[T] = _build_program(T)
    nc = _CACHE[T]
    res = run_bass_kernel_spmd(nc, in_maps, core_ids=list(range(NCORES)))
    out = np.concatenate([res.results[c]["out"] for c in range(NCORES)], axis=0)
    return out.reshape(N_NODES, NUM_HEADS, OUT_DIM).astype(np.float32)


# revision 4
# speedup vs baseline: 1.5587x; 1.3380x over previous
"""Trainium2 Bass kernel for metapath-GRU + GAT-style edge softmax message passing.

v5 — dense-packed edges + minimal instruction count. This runtime charges
~60-130us per instruction on the critical path, so instruction count dominates
wall time; transfer bytes come second (~50MB/s tunnel).

  - Features uploaded SHARDED fp16, AllGathered on device into a 256B-stride
    padded table [20480,128] fp16 (col 64 unused-garbage except where the
    gather's ones-row trick writes); per-768-slot dma_gather (SWDGE, 50
    descriptors/instruction to fit the runtime's ~63-descriptor ring) pulls
    edge-slot features feature-major into SBUF ring tiles of 1536 slots.
  - Edges are packed DENSE (no per-window padding): core k owns nodes
    [2500k,2500k+2500); its edges sorted by dst occupy slots 0..n-1 padded to
    17*768=13056. An e-tile (128 slots) may span two dst windows; it then
    scatters twice with complementary one-hot masks (sentinel 200 marks slots
    outside the target window).
  - GRU runs contiguous 512-wide pieces (26/core); gate biases are folded into
    the x-matmul via a ones-row (gather-table col 64 = 1.0, wihT row 64 =
    bias); DVE arithmetic is 4-chunk-wide ([128, 2048] tiles).
  - Device outputs per-window numerator macc [2500,512] fp16 and denominator
    dacc [WPC*8=160 cols] packed [128,160] f32; the division happens on host.
  - attention exp(leakyrelu) via native ACT Lrelu+Exp.
"""

import sys

sys.path.insert(0, "/opt/trn_rl_repo")

import numpy as np

# ---- problem constants (hardcoded per contract) ----
N_NODES = 20000
N_EDGES = 100000
MP_LEN = 3
OUT_DIM = 64
NUM_HEADS = 8
HID = 512
G3 = 1536
NCORES = 8
NPC = N_NODES // NCORES          # 2500 nodes per core
WPC = (NPC + 127) // 128         # 20 windows per core
LAST_W_ROWS = NPC - 128 * (WPC - 1)  # 68
NPAD = 20480                     # feature table padded to 160*128 rows
SHARD = NPAD // NCORES           # 2560
GSEG = 768                       # slots per dma_gather (50 descriptors)
SEGC = 64                        # idx cols per gather segment (48 used, 64B pad)
NSEG = 17                        # gather segments per hop per core
S = NSEG * GSEG                  # 13056 edge slots per core
NTILE = S // 128                 # 102 e-tiles per core

_CACHE = {}


def _build_program(sig):
    """sig: per-e-tile tuple of (window, dstloc_col, first, last) entries."""
    import concourse.bacc as bacc
    import concourse.tile as tile
    from concourse import mybir
    from concourse import library_config

    f32 = mybir.dt.float32
    f32r = mybir.dt.float32r
    f16 = mybir.dt.float16
    i16 = mybir.dt.int16
    AF = mybir.ActivationFunctionType
    OP = mybir.AluOpType

    tile_entries, ndl = sig

    nc = bacc.Bacc(
        "TRN2", target_bir_lowering=False, debug=False,
        enable_asserts=False, num_devices=NCORES,
    )
    fshard_d = nc.dram_tensor("fshard", [SHARD, 64], f16, kind="ExternalInput").ap()
    idx_d = nc.dram_tensor("idx", [16, 3 * NSEG * SEGC], i16, kind="ExternalInput").ap()
    dstloc = nc.dram_tensor("dstloc", [128, ndl], f32, kind="ExternalInput").ap()
    wihT_d = nc.dram_tensor("wihT", [65, G3], f16, kind="ExternalInput").ap()
    wsm_sh = nc.dram_tensor("wsm", [16, 4 * G3 + 304], f32, kind="ExternalInput").ap()
    out_d = nc.dram_tensor("out", [NPC, HID], f16, kind="ExternalOutput").ap()
    den_d = nc.dram_tensor("den", [128, 8 * WPC], f32, kind="ExternalOutput").ap()

    from contextlib import ExitStack
    with tile.TileContext(nc) as tc, ExitStack() as es:
        nc.gpsimd.load_library(library_config.mlp)

        dram = es.enter_context(tc.tile_pool(name="dram", bufs=1, space="DRAM"))
        cpool = es.enter_context(tc.tile_pool(name="const", bufs=1))
        gp = es.enter_context(tc.tile_pool(name="gp", bufs=2))
        wk = es.enter_context(tc.tile_pool(name="work", bufs=2))
        hp = es.enter_context(tc.tile_pool(name="hp", bufs=2))
        mp = es.enter_context(tc.tile_pool(name="mp", bufs=4))
        op_ = es.enter_context(tc.tile_pool(name="op", bufs=2))
        pg = es.enter_context(tc.tile_pool(name="pg", bufs=1, space="PSUM"))
        pt = es.enter_context(tc.tile_pool(name="pt", bufs=1, space="PSUM"))
        pam = es.enter_context(tc.tile_pool(name="pam", bufs=2, space="PSUM"))
        pad_ = es.enter_context(tc.tile_pool(name="pad", bufs=1, space="PSUM"))

        # ---- collectives ----
        rg = [list(range(NCORES))]
        BP = mybir.AluOpType.bypass

        fb_in = dram.tile([SHARD, 64], f16, name="fb_in")
        fb_out = dram.tile([NPAD, 64], f16, name="fb_out")
        padded = dram.tile([NPAD, 128], f16, name="padded")
        nc.sync.dma_start(out=fb_in[:, :], in_=fshard_d[:, :])
        nc.gpsimd.collective_compute(
            "AllGather", BP, replica_groups=rg,
            ins=[fb_in[:, :].opt()], outs=[fb_out[:, :].opt()])

        wsm_in = dram.tile([16, 4 * G3 + 304], f32, name="wsm_in")
        wsm_out = dram.tile([128, 4 * G3 + 304], f32, name="wsm_out")
        nc.sync.dma_start(out=wsm_in[:, :], in_=wsm_sh[:, :])
        nc.gpsimd.collective_compute(
            "AllGather", BP, replica_groups=rg,
            ins=[wsm_in[:, :].opt()], outs=[wsm_out[:, :].opt()])

        # ---- feature relayout: [NPAD,64] -> padded[:,0:64] (256B stride) ----
        NB = NPAD // 128
        for b in range(NB):
            nc.sync.dma_start(out=padded[b * 128:(b + 1) * 128, 0:64],
                              in_=fb_out[b * 128:(b + 1) * 128, :])

        # ---- weights into SBUF ----
        wihT = cpool.tile([65, G3], f16, name="wihT_sb")
        nc.sync.dma_start(out=wihT[:, :], in_=wihT_d[:, :])
        whh = cpool.tile([128, 4 * G3], f32r, name="whh_sb")
        nc.sync.dma_start(out=whh[:, :], in_=wsm_out[:, 0:4 * G3].bitcast(f32r))
        smalls = cpool.tile([128, 304], f32, name="smalls_sb")
        nc.sync.dma_start(out=smalls[:, :], in_=wsm_out[:, 4 * G3:4 * G3 + 304])
        amat = smalls[:, 0:32].bitcast(f32r)
        bias = smalls[:, 32:48]
        iota = smalls[:, 48:176]
        ident = smalls[:, 176:304].bitcast(f32r)

        def b_hn(j):
            return bias[:, 12 + j:13 + j]

        # dstloc, transposed on host: one DMA
        dlT = cpool.tile([128, ndl], f32, name="dlT")
        nc.sync.dma_start(out=dlT[:, :], in_=dstloc[:, :])

        # ---- indices into SBUF (8x replicated over 16-partition groups) ----
        idxs = cpool.tile([128, 3 * NSEG * SEGC], i16, name="idx_sb")
        for k in range(8):
            nc.sync.dma_start(out=idxs[16 * k:16 * (k + 1), :], in_=idx_d[:, :])

        # denominators for ALL windows live in one PSUM bank; each window's
        # start=True wipes the whole 2KB bank (PSUM zero-region granularity),
        # so every window's slice is staged to SBUF right after its stop.
        dacc = pad_.tile([128, 8 * WPC], f32, name="dacc_all")
        nc.vector.memset(dacc[:, :], 0.0)
        den_sb = cpool.tile([128, 8 * WPC], f32, name="den_sb")

        # b_hn broadcast wide
        bhn_w = cpool.tile([128, 4 * 512], f32, name="bhn_w")
        nc.vector.memset(bhn_w[:, :], 0.0)
        for j in range(4):
            nc.vector.tensor_scalar(bhn_w[:, j * 512:(j + 1) * 512],
                                    bhn_w[:, j * 512:(j + 1) * 512],
                                    b_hn(j), None, op0=OP.add)

        def gather_group(gi, nseg):
            """Gather segments [2*gi, 2*gi+nseg) for all 3 hops into one tile."""
            gt3 = []
            for t in range(3):
                gt = gp.tile([128, 1, 2 * GSEG], f16, name=f"g{t}_{gi}", tag=f"g{t}")
                for h in range(nseg):
                    seg = gi * 2 + h
                    i0 = (t * NSEG + seg) * SEGC
                    nc.gpsimd.dma_gather(
                        gt[:, :, h * GSEG:(h + 1) * GSEG], padded[:, :],
                        idxs[:, i0:i0 + GSEG // 16],
                        GSEG, GSEG, 128, transpose=True)
                nc.vector.memset(gt[64:65, 0, :], 1.0)
                gt3.append(gt)
            return gt3

        def wih_slice(gate, j):
            o = gate * HID + j * 128
            return wihT[:, o:o + 128]

        def whh_slice(k, gate, j):
            o = k * G3 + gate * HID + j * 128
            return whh[:, o:o + 128]

        macc_of = {}

        def emit_out_window(w):
            rows = 128 if w < WPC - 1 else LAST_W_ROWS
            macc = macc_of.pop(w)
            osb = op_.tile([128, HID], f16, name=f"osb{w}", tag="osb")
            nc.scalar.activation(osb[:, :], macc[:, :], AF.Copy)
            nc.sync.dma_start(out=out_d[w * 128:w * 128 + rows, :], in_=osb[:rows, :])

        NGRP = (NSEG + 1) // 2
        ti_base = 0
        for gi in range(NGRP):
            nseg = 2 if gi < NGRP - 1 or NSEG % 2 == 0 else 1
            gcur = gather_group(gi, nseg)
            gslots = nseg * GSEG
            pieces = ([(i * 512, 512) for i in range(gslots // 512)]
                      if gslots % 512 == 0 else [(0, 512), (512, 256)])
            for (off, cw) in pieces:
                xs = [gcur[t][0:65, 0, off:off + cw] for t in range(3)]
                # ---- GRU (wide DVE) ----
                h_w = None
                for step in range(3):
                    xt = xs[step]
                    psr, psz, psn, pshn = [], [], [], []
                    for j in range(4):
                        pr = pg.tile([128, cw], f32, name=f"psr{gi}{off}{step}{j}", tag="r")
                        pz = pg.tile([128, cw], f32, name=f"psz{gi}{off}{step}{j}", tag="z")
                        pn = pg.tile([128, cw], f32, name=f"psn{gi}{off}{step}{j}", tag="nn")
                        if step == 0:
                            nc.tensor.matmul(pr[:, :], wih_slice(0, j), xt, start=True, stop=True)
                            nc.tensor.matmul(pz[:, :], wih_slice(1, j), xt, start=True, stop=True)
                            nc.tensor.matmul(pn[:, :], wih_slice(2, j), xt, start=True, stop=True)
                        else:
                            nc.tensor.matmul(pr[:, :], wih_slice(0, j), xt, start=True, stop=False)
                            nc.tensor.matmul(pz[:, :], wih_slice(1, j), xt, start=True, stop=False)
                            for k in range(4):
                                hk = h_w[:, k * 512:k * 512 + cw]
                                nc.tensor.matmul(pr[:, :], whh_slice(k, 0, j), hk,
                                                 start=False, stop=(k == 3))
                                nc.tensor.matmul(pz[:, :], whh_slice(k, 1, j), hk,
                                                 start=False, stop=(k == 3))
                            nc.tensor.matmul(pn[:, :], wih_slice(2, j), xt, start=True, stop=True)
                            ph = pg.tile([128, cw], f32, name=f"pshn{gi}{off}{step}{j}", tag="hn")
                            for k in range(4):
                                nc.tensor.matmul(ph[:, :], whh_slice(k, 2, j),
                                                 h_w[:, k * 512:k * 512 + cw],
                                                 start=(k == 0), stop=(k == 3))
                            pshn.append(ph)
                        psr.append(pr)
                        psz.append(pz)
                        psn.append(pn)

                    def ws(tile_, j):
                        return tile_[:, j * 512:j * 512 + cw]

                    r_w = wk.tile([128, 4 * 512], f32, name=f"rw{gi}{off}{step}", tag="r_w")
                    z_w = wk.tile([128, 4 * 512], f32, name=f"zw{gi}{off}{step}", tag="z_w")
                    hn_w = wk.tile([128, 4 * 512], f32, name=f"hw{gi}{off}{step}", tag="hn_w")
                    t2_w = wk.tile([128, 4 * 512], f32, name=f"tw{gi}{off}{step}", tag="t2_w")
                    for j in range(4):
                        nc.scalar.activation(ws(r_w, j), psr[j][:, :], AF.Sigmoid)
                        nc.scalar.activation(ws(z_w, j), psz[j][:, :], AF.Sigmoid)
                    if step == 0:
                        nc.vector.tensor_tensor(hn_w[:, :], r_w[:, :], bhn_w[:, :], op=OP.mult)
                    else:
                        for j in range(4):
                            nc.vector.tensor_scalar(ws(hn_w, j), pshn[j][:, :], b_hn(j), None, op0=OP.add)
                        nc.vector.tensor_tensor(hn_w[:, :], r_w[:, :], hn_w[:, :], op=OP.mult)
                    for j in range(4):
                        nc.vector.tensor_tensor(ws(t2_w, j), psn[j][:, :], ws(hn_w, j), op=OP.add)
                    nc.scalar.activation(hn_w[:, :], t2_w[:, :], AF.Tanh)
                    hn_new = hp.tile([128, 4 * 512], f32r, name=f"h{gi}{off}{step}",
                                     tag=f"h{step % 2}")
                    if step == 0:
                        nc.vector.tensor_tensor(t2_w[:, :], z_w[:, :], hn_w[:, :], op=OP.mult)
                        nc.vector.tensor_tensor(hn_new[:, :], hn_w[:, :], t2_w[:, :], op=OP.subtract)
                    else:
                        nc.vector.tensor_tensor(t2_w[:, :], h_w[:, :], hn_w[:, :], op=OP.subtract)
                        nc.vector.tensor_tensor(t2_w[:, :], z_w[:, :], t2_w[:, :], op=OP.mult)
                        nc.vector.tensor_tensor(hn_new[:, :], hn_w[:, :], t2_w[:, :], op=OP.add)
                    h_w = hn_new
                # ---- attention logits -> exp(leakyrelu) ----
                psa = pg.tile([8, cw], f32, name=f"psa{gi}{off}", tag="nn")
                for k in range(4):
                    nc.tensor.matmul(psa[:, :], amat[:, k * 8:(k + 1) * 8],
                                     h_w[:, k * 512:k * 512 + cw],
                                     start=(k == 0), stop=(k == 3))
                lr = wk.tile([8, cw], f32, name=f"lr{gi}{off}", tag="lr")
                nc.scalar.activation(lr[:, :], psa[:, :], AF.Lrelu, alpha=0.01)
                eaT = wk.tile([8, cw], f32r, name=f"eaT{gi}{off}", tag="eaT")
                nc.scalar.activation(eaT[:, :], lr[:, :], AF.Exp)
                # ---- per e-tile: transpose, ea-mul, scatter (1-2 windows) ----
                for et in range(cw // 128):
                    ti = ti_base + off // 128 + et
                    ees = et * 128
                    pse = pt.tile([128, 8], f32r, name=f"pse{ti}", tag="tp")
                    nc.tensor.transpose(pse[:, :], eaT[:, ees:ees + 128], ident[:8, :8])
                    ea_em = mp.tile([128, 8], f32r, name=f"eaem{ti}", tag="ea_em")
                    nc.scalar.activation(ea_em[:, :], pse[:, :], AF.Copy)
                    msg = mp.tile([128, HID], f32r, name=f"msg{ti}", tag="msg")
                    for j in range(4):
                        pst = pt.tile([128, 128], f32r, name=f"pst{ti}{j}", tag="tp")
                        nc.tensor.transpose(pst[:, :], h_w[:, j * 512 + ees:j * 512 + ees + 128],
                                            ident[:, :])
                        for hh in range(2):
                            hd = 2 * j + hh
                            nc.vector.tensor_scalar(
                                msg[:, hd * 64:(hd + 1) * 64], pst[:, hh * 64:(hh + 1) * 64],
                                ea_em[:, hd:hd + 1].bitcast(f32), None, op0=OP.mult)
                    for (w, col, first, last) in tile_entries[ti]:
                        if first:
                            macc_of[w] = pam.tile([128, HID], f32, name=f"macc{w}", tag="macc")
                        macc = macc_of[w]
                        ohs = mp.tile([128, 128], f32r, name=f"ohs{ti}_{w}", tag="ohs")
                        nc.vector.tensor_scalar(ohs[:, :], iota[:, :], dlT[:, col:col + 1],
                                                None, op0=OP.is_equal)
                        nc.tensor.matmul(macc[:, :], ohs[:, :], msg[:, :],
                                         start=first, stop=last, skip_group_check=True)
                        nc.tensor.matmul(dacc[:, 8 * w:8 * w + 8], ohs[:, :], ea_em[:, :],
                                         start=False, stop=last, skip_group_check=True)
                        if last:
                            emit_out_window(w)
            ti_base += gslots // 128
        # ship all denominators once (dacc accumulates start=False onto the
        # initial memset-zeros, so no start=True ever wipes the shared bank)
        nc.scalar.activation(den_sb[:, :], dacc[:, :], AF.Copy)
        nc.sync.dma_start(out=den_d[:, :], in_=den_sb[:, :])

    nc.compile()
    return nc


def _preprocess(features, W_ih, W_hh, b_ih, b_hh, attn, idx, dst):
    feats = np.asarray(features, np.float32)
    idx = np.asarray(idx).astype(np.int64)
    dst = np.asarray(dst).astype(np.int64)
    order = np.argsort(dst, kind="stable")
    ds = dst[order]
    idxs = idx[order]
    core_of = ds // NPC
    local = ds % NPC
    nloc = local % 128
    win = local // 128

    core_cnt = np.bincount(core_of, minlength=NCORES)
    assert core_cnt.max() <= S, "edge slots overflow; raise NSEG"
    core_start = np.zeros(NCORES, np.int64)
    core_start[1:] = np.cumsum(core_cnt)[:-1]
    core_slot = np.arange(N_EDGES) - core_start[core_of]

    # hop indices, segmented per 768 slots with 64-col alignment padding
    idx_all = np.zeros((NCORES, 3, NSEG, SEGC * 16), np.int16)
    seg_of = core_slot // GSEG
    pos_in_seg = core_slot % GSEG
    for t in range(3):
        idx_all[core_of, t, seg_of, pos_in_seg] = idxs[:, t].astype(np.int16)
    cols = 3 * NSEG * SEGC
    idx_w = np.ascontiguousarray(
        idx_all.reshape(NCORES, cols, 16).transpose(0, 2, 1))

    # unioned window->tile spans (identical SPMD program across cores):
    # window w covers tiles [min_c t0, max_c t1]; cores with no edges of w in
    # a tile supply an all-sentinel dstloc column.
    t0_u = np.full(WPC, NTILE, np.int64)
    t1_u = np.full(WPC, -1, np.int64)
    for c in range(NCORES):
        sel = core_of == c
        cs = core_slot[sel]
        cw_ = win[sel]
        for w in range(WPC):
            m = cw_ == w
            if not m.any():
                continue
            t0_u[w] = min(t0_u[w], int(cs[m].min()) // 128)
            t1_u[w] = max(t1_u[w], int(cs[m].max()) // 128)
    entries = [[] for _ in range(NTILE)]
    col_of = {}
    ndl = 0
    for w in range(WPC):
        assert t1_u[w] >= 0, f"window {w} empty on all cores"
        for ti in range(t0_u[w], t1_u[w] + 1):
            col_of[(w, ti)] = ndl
            entries[ti].append((w, ndl, ti == t0_u[w], ti == t1_u[w]))
            ndl += 1
    assert max(len(e) for e in entries) <= 2, "window overlap depth > 2"
    tile_entries = tuple(tuple(e) for e in entries)

    dl_arr_ = np.full((NCORES, 128, ndl), 200.0, np.float32)
    tile_of_slot = core_slot // 128
    col_idx = np.array([col_of.get((int(w_), int(t_)), -1)
                        for w_, t_ in zip(win, tile_of_slot)])
    assert (col_idx >= 0).all()
    dl_arr_[core_of, core_slot % 128, col_idx] = nloc

    sig = (tile_entries, ndl)
    dl_arr = dl_arr_

    # padded fp16 feature table, sharded
    fpad = np.zeros((NPAD, 64), np.float16)
    fpad[:N_NODES] = feats.astype(np.float16)

    W_ih = np.asarray(W_ih, np.float32)
    W_hh = np.asarray(W_hh, np.float32)
    b_ih = np.asarray(b_ih, np.float32)
    b_hh = np.asarray(b_hh, np.float32)
    attn = np.asarray(attn, np.float32)
    b_rz_full = b_ih + b_hh
    brow = np.concatenate([b_rz_full[0:HID], b_rz_full[HID:2 * HID],
                           b_ih[2 * HID:3 * HID]])  # [1536]
    wihT = np.concatenate([W_ih.T, brow[None, :]], axis=0).astype(np.float16)
    whhT = W_hh.T  # [512, 1536]
    whh6 = np.concatenate([whhT[k * 128:(k + 1) * 128, :] for k in range(4)], axis=1)
    bias16 = np.zeros((128, 16), np.float32)
    for j in range(4):
        bias16[:, j] = b_rz_full[j * 128:(j + 1) * 128]
        bias16[:, 4 + j] = b_rz_full[HID + j * 128:HID + (j + 1) * 128]
        bias16[:, 8 + j] = b_ih[2 * HID + j * 128:2 * HID + (j + 1) * 128]
        bias16[:, 12 + j] = b_hh[2 * HID + j * 128:2 * HID + (j + 1) * 128]
    amat = np.zeros((HID, 8), np.float32)
    for h in range(8):
        amat[h * 64:(h + 1) * 64, h] = attn[h]
    amat32 = np.zeros((128, 32), np.float32)
    for k in range(4):
        amat32[:, k * 8:(k + 1) * 8] = amat[k * 128:(k + 1) * 128, :]
    ident = np.eye(128, dtype=np.float32)
    iota = np.tile(np.arange(128, dtype=np.float32)[None, :], (128, 1))
    smalls = np.concatenate([amat32, bias16, iota, ident], axis=1)  # [128, 304]
    wsm = np.ascontiguousarray(
        np.concatenate([whh6.astype(np.float32), smalls], axis=1))

    in_maps = []
    for c in range(NCORES):
        in_maps.append({
            "fshard": np.ascontiguousarray(fpad[c * SHARD:(c + 1) * SHARD]),
            "idx": np.ascontiguousarray(idx_w[c]),
            "dstloc": np.ascontiguousarray(dl_arr[c]),
            "wihT": wihT,
            "wsm": np.ascontiguousarray(wsm[c * 16:(c + 1) * 16]),
        })
    return sig, in_maps


def kernel(**inputs):
    from concourse.bass_utils import run_bass_kernel_spmd

    sig, in_maps = _preprocess(
        inputs["features"], inputs["W_ih"], inputs["W_hh"], inputs["b_ih"],
        inputs["b_hh"], inputs["attn"], inputs["edge_metapath_indices"],
        inputs["edge_dst"])
    if sig not in _CACHE:
        _CACHE[sig] = _build_program(sig)
    nc = _CACHE[sig]
    res = run_bass_kernel_spmd(nc, in_maps, core_ids=list(range(NCORES)))
    outs = []
    for c in range(NCORES):
        num = res.results[c]["out"].astype(np.float32)      # [2500, 512]
        den = res.results[c]["den"]                         # [128, 160]
        den_full = (den.T.reshape(WPC, 8, 128).transpose(0, 2, 1)
                    .reshape(WPC * 128, 8)[:NPC])
        den_full = np.where(den_full > 1e-30, den_full, 1.0)
        outs.append(num.reshape(NPC, 8, 64) / den_full[:, :, None])
    out = np.concatenate(outs, axis=0)
    return out.reshape(N_NODES, NUM_HEADS, OUT_DIM).astype(np.float32)
